# revision 28
# baseline (speedup 1.0000x reference)
"""GateRow kernel for Trainium2 (8 NeuronCores, SPMD gate-sharded).

Problem: out[b, g] = gates[g, 2*x[b, c0[g]] + x[b, c1[g]]]
  x: [16384, 8192] bool, gates: [8192, 4] bool, choices: [8192, 2] int32.

Strategy (per core, gate-sharded GPC=1024, batch bit-packed 8 bits/byte):
  host:  TAB = [packbits(x)^T ; ~packbits(x)^T ; ones ; zeros]
         (16386 rows x 2048 bytes).  Classify each gate:
           AND/OR class (14/16 truth tables): f = (a' & b') ^ m,
             a'/b' TAB rows, m per-gate constant byte mask (0x00/0xFF)
           XOR class (tt 0110/1001): f = a' ^ b'
         XOR-class gates are moved to dedicated tail slots per core.
  device:
    normal slots:  dma_gather a'|b' rows -> q = a'&b' -> o = q ^ mask
                   (mask via stride-0 broadcast of a per-partition u32)
    tail slots:    dma_gather a'|b' rows -> o = a'^b'
    all bitwise ops as uint32 lanes on DVE; packed rows DMA'd out.
  host:  unpack bits + transpose to [B, G] bool.

Every core holds exactly 768 normal + 256 tail gates (degenerate
single-operand/constant gates are moved to the tail as `row ^ zeros`),
so the 8 slots are exactly full: 2048 gather descriptors per core with
zero padding (~4 MB gather + 2 MB packed out).  SWDGE descriptor
generation and DMA-ring descriptor processing both scale with
descriptor count, which is what the call plan minimizes; the last call
is a single tail slot so the final transfer->XOR->out chain is short,
and its all-zero b' rows are trailing -1 entries (no descriptor) with
the tile slot pre-memset instead.  Measured ~46 us on hardware.
"""

import sys

for _p in ("/opt/trn_rl_repo", "/opt/pypackages"):
    if _p not in sys.path:
        sys.path.append(_p)

from contextlib import ExitStack
from itertools import product

import numpy as np

import concourse.bass as bass  # noqa: F401
import concourse.bacc as bacc
import concourse.tile as tile
import concourse.mybir as mybir
from concourse.bass_utils import run_bass_kernel_spmd

B, N, G, NCORES = 16384, 8192, 8192, 8
GPC = G // NCORES      # 1024 gates per core
BPACK = B // 8         # 2048 packed bytes per table row
ROWS = 2 * N + 2       # x rows, ~x rows, ones, zeros
ZROW = 2 * N + 1       # all-zeros row (pad target)

# XOR-expressible truth tables: f = a' ^ b' with rows from TAB
#   {tt: (sa, sb)}; codes 0=a,1=~a,2=b,3=~b,4=ones,5=zeros


def _xor_forms():
    def val(sel, a, b):
        return [a, 1 - a, b, 1 - b, 1, 0][sel]

    forms = {}
    for tt in range(16):
        # prefer sb=5 (zeros row): the tail-B trim relies on degenerate
        # gates having the zeros row on the b' side
        for sb, sa in product((5, 2, 3, 0, 1, 4), range(6)):
            if all(
                (val(sa, a, b) ^ val(sb, a, b)) == ((tt >> (2 * a + b)) & 1)
                for a in (0, 1)
                for b in (0, 1)
            ):
                forms[tt] = (sa, sb)
                break
    return forms


_XFORMS = _xor_forms()

# ---------------------------------------------------------------------------
# Gate classification
#   AND/OR class: f(a,b) = (a' & b') ^ m,  a' in {a,~a,1,0}, b' in {b,~b,1,0}
#   XOR class (tt 6/9): f = a' ^ b'
#   selector codes: 0 = a, 1 = ~a, 2 = b, 3 = ~b, 4 = ones, 5 = zeros
# ---------------------------------------------------------------------------


def _classify_gates():
    def val(sel, a, b):
        return [a, 1 - a, b, 1 - b, 1, 0][sel]

    forms = np.full((16, 3), -1, dtype=np.int64)  # (sa, sb, m) ; m==2 -> XOR class
    for tt in range(16):
        if tt in (6, 9):
            # a ^ b   /   ~a ^ b
            forms[tt] = (0 if tt == 6 else 1, 2, 2)
            continue
        found = False
        for sa, sb, m in product([0, 1, 4, 5], [2, 3, 4, 5], [0, 1]):
            if all(
                ((val(sa, a, b) & val(sb, a, b)) ^ m) == ((tt >> (2 * a + b)) & 1)
                for a in (0, 1)
                for b in (0, 1)
            ):
                forms[tt] = (sa, sb, m)
                found = True
                break
        assert found, f"truth table {tt} not representable"
    return forms


_FORMS = _classify_gates()

# ---------------------------------------------------------------------------
# Device program.  call_plan: list of (slot_start, n_slots, num_idxs_reg,
# is_tail); nslot_n / nslot_x fixed by the plan.
# ---------------------------------------------------------------------------


def build_nc(call_plan, nslot_total, nslot_n):
    u32 = mybir.dt.uint32

    nc = bacc.Bacc(
        "TRN2",
        target_bir_lowering=False,
        debug=False,
        num_devices=NCORES,
    )
    tab = nc.dram_tensor("tab", [ROWS, BPACK], mybir.dt.uint8, kind="ExternalInput")
    total_idx_cols = sum(
        (128 if kind in (2, 3) else 2 * n * 128) // 16
        for _, n, _, kind in call_plan
    )
    idxs = nc.dram_tensor(
        "idxs", [128, total_idx_cols], mybir.dt.int16, kind="ExternalInput"
    )
    cst = nc.dram_tensor("cst", [128, nslot_n, 4], mybir.dt.uint8, kind="ExternalInput")
    outd = nc.dram_tensor(
        "out", [128, nslot_total * BPACK], mybir.dt.uint8, kind="ExternalOutput"
    )

    with tile.TileContext(nc) as tc, ExitStack() as ctx:
        pconst = ctx.enter_context(tc.tile_pool(name="const", bufs=1))
        pg = ctx.enter_context(tc.tile_pool(name="gather", bufs=1))
        pq = ctx.enter_context(tc.tile_pool(name="and", bufs=1))
        po = ctx.enter_context(tc.tile_pool(name="out", bufs=1))

        idx_t = pconst.tile([128, total_idx_cols], mybir.dt.int16)
        nc.sync.dma_start(idx_t[:], idxs[:])
        cst_t = pconst.tile([128, nslot_n, 4], mybir.dt.uint8)
        nc.sync.dma_start(cst_t[:], cst[:])

        icol = 0
        half_tiles = {}
        for k, (s0, n, nreg, kind) in enumerate(call_plan):
            num_idxs = 128 if kind in (2, 3) else 2 * n * 128
            ncols = num_idxs // 16
            if kind == 2:
                g_t = pg.tile([128, 2, BPACK], mybir.dt.uint8, tag=f"g{k}")
                half_tiles[s0] = g_t
                dst = g_t[:, 0:1, :]
            elif kind == 3:
                g_t = half_tiles[s0]
                dst = g_t[:, 1:2, :]
            else:
                g_t = pg.tile([128, 2 * n, BPACK], mybir.dt.uint8, tag=f"g{k}")
                dst = g_t[:]
            if nreg < num_idxs:
                # trimmed b'-side entries are never written by the gather
                nc.vector.memset(g_t[:, n : 2 * n, :].bitcast(mybir.dt.uint32), 0)
            nc.gpsimd.dma_gather(
                dst,
                tab[:],
                idx_t[:, icol : icol + ncols],
                num_idxs,
                nreg,
                BPACK,
                single_packet=False,
            )
            icol += ncols
            if kind == 2:
                continue  # compute happens after the B-half arrives
            o_t = po.tile([128, n, BPACK], mybir.dt.uint8, tag=f"o{k}")
            if kind == 1:
                nc.vector.tensor_tensor(
                    o_t[:].bitcast(u32),
                    g_t[:, 0:n, :].bitcast(u32),
                    g_t[:, n : 2 * n, :].bitcast(u32),
                    mybir.AluOpType.bitwise_xor,
                )
            else:
                q_t = pq.tile([128, n, BPACK], mybir.dt.uint8, tag=f"q{k}")
                nc.vector.tensor_tensor(
                    q_t[:].bitcast(u32),
                    g_t[:, 0:n, :].bitcast(u32),
                    g_t[:, n : 2 * n, :].bitcast(u32),
                    mybir.AluOpType.bitwise_and,
                )
                nc.vector.tensor_tensor(
                    o_t[:].bitcast(u32),
                    q_t[:].bitcast(u32),
                    cst_t[:, s0 : s0 + n, :]
                    .bitcast(u32)
                    .broadcast_to([128, n, BPACK // 4]),
                    mybir.AluOpType.bitwise_xor,
                )
            nc.sync.dma_start(
                outd[:, s0 * BPACK : (s0 + n) * BPACK], o_t[:]
            )
    nc.compile()
    return nc


# ---------------------------------------------------------------------------
# Host-side input prep / output assembly
# ---------------------------------------------------------------------------


def _prep(x, gates, choices):
    x8 = np.asarray(x, dtype=np.uint8)
    g8 = np.asarray(gates, dtype=np.uint8)
    ch = np.asarray(choices, dtype=np.int64)

    xp = np.packbits(x8, axis=0, bitorder="little")  # [B/8, N]
    tabx = np.ascontiguousarray(xp.T)                # [N, BPACK]
    tab_full = np.empty((ROWS, BPACK), dtype=np.uint8)
    tab_full[:N] = tabx
    tab_full[N : 2 * N] = tabx ^ 0xFF
    tab_full[2 * N] = 0xFF
    tab_full[ZROW] = 0

    tt = (g8[:, 0] | (g8[:, 1] << 1) | (g8[:, 2] << 2) | (g8[:, 3] << 3)).astype(
        np.int64
    )
    sel = _FORMS[tt]                       # [G, 3] (sa, sb, m|2)
    c0, c1 = ch[:, 0], ch[:, 1]

    def row_of(code):
        return np.select(
            [code == 0, code == 1, code == 2, code == 3, code == 4, code == 5],
            [c0, N + c0, c1, N + c1,
             np.full(G, 2 * N, np.int64), np.full(G, ZROW, np.int64)],
        )

    rowA = row_of(sel[:, 0])
    rowB = row_of(sel[:, 1])
    is_x = sel[:, 2] == 2
    mask = (sel[:, 2] == 1).astype(np.uint8) * 0xFF

    # XOR-form rows (valid for tts in _XFORMS): used for gates placed in the
    # tail region, including degenerate gates moved there for rebalancing.
    xsel = np.zeros((G, 2), np.int64)
    can_x = np.zeros(G, bool)
    for t, (sa, sb) in _XFORMS.items():
        m = tt == t
        xsel[m] = (sa, sb)
        can_x[m] = True
    xrowA = row_of(xsel[:, 0])
    xrowB = row_of(xsel[:, 1])

    # per-core gate lists: move exactly (256 - n_xor) degenerate
    # (XOR-expressible) gates into the tail region so every core has
    # exactly 768 normal + 256 tail gates -- 8 slots, zero padding.
    nslot_n, nslot_x = 6, 2
    norm_ids, xor_ids = [], []
    for c in range(NCORES):
        gl = np.arange(c * GPC, (c + 1) * GPC)
        nrm = gl[~is_x[gl]]
        xr = list(gl[is_x[gl]])
        spill = GPC - nslot_n * 128 - len(xr)
        assert spill >= 0, "more XOR-class gates than tail capacity"
        if spill > 0:
            movable = nrm[can_x[nrm]]
            assert len(movable) >= spill, "not enough degenerate gates to move"
            mv = set(movable[:spill].tolist())
            # XOR-class gates first, moved degenerates (b' = zeros row) last:
            # the tail-B call can then drop its b'-side zero-row gathers
            xr = sorted(xr) + sorted(mv)
            nrm = np.array([g for g in nrm if g not in mv])
        norm_ids.append(nrm)
        xor_ids.append(np.array(xr, dtype=np.int64))
        assert len(nrm) == nslot_n * 128 and len(xor_ids[-1]) == nslot_x * 128
    nn = np.array([len(v) for v in norm_ids])
    nx = np.array([len(v) for v in xor_ids])
    nslot_total = nslot_n + nslot_x

    # call plan (shared across cores): small first call so transfers start
    # early; a 1-slot tail call last so the final transfer->DVE->out chain
    # is a single XOR op + 256 KB write.  The last tail call holds mostly
    # moved degenerates whose b' is the zeros row: those b' entries are
    # trailing -1 (no descriptor) and the b'-tile slot is pre-memset to 0.
    nx_orig = np.array(
        [is_x[c * GPC : (c + 1) * GPC].sum() for c in range(NCORES)]
    )
    nxb_max = int(max(0, (nx_orig - 128).max()))
    # kind: 0 = normal, 1 = tail, 2 = A-half of slot, 3 = B-half of slot.
    # Slot 0 is gathered as two 128-descriptor halves so the first
    # descriptor-gen (which gates the whole DMA pipeline) is minimal.
    call_plan = [
        (0, 1, 128, 2),
        (0, 1, 128, 3),
        (nslot_n, 1, 2 * 128, 1),
        (1, 2, 2 * 2 * 128, 0),
        (3, 2, 2 * 2 * 128, 0),
        (5, 1, 2 * 128, 0),
        (nslot_n + 1, 1, 128 + nxb_max, 1),
    ]

    in_maps = []
    for c in range(NCORES):
        na, nxr = nn[c], nx[c]
        # per-position rows for the reordered layout
        posA = np.full(nslot_n * 128, ZROW, np.int64)
        posB = np.full(nslot_n * 128, ZROW, np.int64)
        posA[:na] = rowA[norm_ids[c]]
        posB[:na] = rowB[norm_ids[c]]
        capx = nslot_x * 128
        xposA = np.full(capx, ZROW, np.int64)
        xposB = np.full(capx, ZROW, np.int64)
        xposA[:nxr] = xrowA[xor_ids[c]]
        xposB[:nxr] = xrowB[xor_ids[c]]

        cols = []
        for s0, n, nreg, kind in call_plan:
            cap = n * 128
            if kind == 1:
                a, b = xposA, xposB
                lo = (s0 - nslot_n) * 128
            else:
                a, b = posA, posB
                lo = s0 * 128
            if kind == 2:
                flat = a[lo : lo + cap].copy()
            elif kind == 3:
                flat = b[lo : lo + cap].copy()
            else:
                flat = np.concatenate([a[lo : lo + cap], b[lo : lo + cap]])
                if nreg < 2 * cap:  # trimmed b'-side: trailing -1, no desc
                    flat[nreg:] = -1
            wrapped = flat.astype(np.int16).reshape(-1, 16).T
            cols.append(np.tile(wrapped, (8, 1)))
        idxs_np = np.ascontiguousarray(np.concatenate(cols, axis=1))

        mcol = np.zeros(nslot_n * 128, np.uint8)
        mcol[:na] = mask[norm_ids[c]]
        cst_np = np.repeat(
            mcol.reshape(nslot_n, 128).T[:, :, None], 4, axis=2
        )  # [128, nslot_n, 4]
        in_maps.append(
            {
                "tab": tab_full,
                "idxs": idxs_np,
                "cst": np.ascontiguousarray(cst_np),
            }
        )

    meta = {
        "call_plan": tuple(call_plan),
        "nslot_n": nslot_n,
        "nslot_total": nslot_total,
        "norm_ids": norm_ids,
        "xor_ids": xor_ids,
    }
    return in_maps, meta


def _assemble(results, meta):
    nslot_n = meta["nslot_n"]
    nslot_total = meta["nslot_total"]
    packed = np.empty((G, BPACK), np.uint8)
    for c in range(NCORES):
        o = results[c]["out"]  # [128, nslot_total*BPACK]
        pos = (
            o.reshape(128, nslot_total, BPACK).transpose(1, 0, 2).reshape(-1, BPACK)
        )
        nids, xids = meta["norm_ids"][c], meta["xor_ids"][c]
        packed[nids] = pos[: len(nids)]
        packed[xids] = pos[nslot_n * 128 : nslot_n * 128 + len(xids)]
    return np.unpackbits(packed, axis=1, bitorder="little").T.astype(bool)


# ---------------------------------------------------------------------------
# Entry point
# ---------------------------------------------------------------------------

_NC_CACHE = {}


def _get_nc(call_plan, nslot_total, nslot_n):
    key = (tuple(call_plan), nslot_total, nslot_n)
    if key not in _NC_CACHE:
        _NC_CACHE[key] = build_nc(call_plan, nslot_total, nslot_n)
    return _NC_CACHE[key]


def kernel(x, gates, choices):
    in_maps, meta = _prep(x, gates, choices)
    nc = _get_nc(meta["call_plan"], meta["nslot_total"], meta["nslot_n"])
    res = run_bass_kernel_spmd(nc, in_maps, list(range(NCORES)))
    return _assemble(res.results, meta)
